# revision 6
# baseline (speedup 1.0000x reference)
# Contrastive (NT-Xent style) loss kernel for 8 Trainium2 NeuronCores.
#
# Math: with z = concat(z_i, z_j)  (N=8192 rows, D=128), zn = row-normalized z,
# sim = (zn @ zn.T)/TEMP, the reference loss reduces exactly to
#   loss = (1/N) * sum_r [ log( sum_{c != r} exp(sim[r,c]) ) - sim[r, (r+B) % N] ]
# (verified bit-for-bit against the reference's mask/gather formulation).
#
# Sharding: data-parallel over rows. Core m receives z rolled by -1024*m rows
# (plus the same data pre-transposed in bf16), so every core runs the IDENTICAL
# program on "its" rows 0..1023: rotation makes the diagonal / positive-pair
# columns core-independent (the positive partner of rotated row r is rotated
# column (r + 4096) % 8192 on every core).
# Per core: row norms on VectorE, rnorm = exp(-0.5*ln(nrm2)) on ScalarE,
# column-normalized zn in bf16, then for each of 8 row-tiles the full
# 128x8192 sim strip via TensorE (bf16 in, fp32 acc) and exp row-sums via
# ScalarE activation-accumulate; the raw diagonal and positive dots are tapped
# from PSUM; per-row loss terms log(rowsum - exp(2*diag)) - 2*pos go to DRAM.
# The host sums the 8x(128x8) per-row terms and divides by N.

import numpy as np

B = 4096
D = 128
N = 2 * B
TEMP = 0.5
NCORES = 8
RPC = N // NCORES          # rows per core = 1024
NT = N // 128              # 64 column tiles of 128
RT = RPC // 128            # 8 row tiles per core
GW = 2048                  # PSUM group width (4 banks), 4 groups per row-tile
NG = N // GW               # 4

_CACHE = {}


def _build():
    import concourse.bass as bass
    import concourse.bacc as bacc
    import concourse.tile as tile
    from concourse import mybir

    f32 = mybir.dt.float32
    bf16 = mybir.dt.bfloat16
    Alu = mybir.AluOpType
    Act = mybir.ActivationFunctionType

    nc = bacc.Bacc(
        "TRN2",
        target_bir_lowering=False,
        debug=False,
        enable_asserts=False,
        num_devices=NCORES,
    )
    zr_d = nc.dram_tensor("zr", [N, D], bf16, kind="ExternalInput").ap()    # rows
    zt_d = nc.dram_tensor("zt", [D, N], bf16, kind="ExternalInput").ap()    # pre-transposed
    identf_d = nc.inline_tensor(np.eye(128, dtype=np.float32), name="identf").ap()
    out_d = nc.dram_tensor("out", [128, RT], f32, kind="ExternalOutput").ap()
    # internal DRAM scratch used to flatten rnorm from (128,64) to row-order (8192,)
    rnsc = nc.dram_tensor("rnsc", [NT, 128], bf16)

    zrv = zr_d.rearrange("(t p) d -> t p d", p=128)  # (64, 128, 128)

    with tile.TileContext(nc) as tc:
        with (
            tc.tile_pool(name="persist", bufs=1) as P,
            tc.tile_pool(name="work", bufs=3) as W,
            tc.tile_pool(name="grp", bufs=2, space="PSUM") as G,
        ):
            zb = P.tile([128, NT, D], bf16)      # raw z rows (norm input)
            zt = P.tile([128, NT, 128], bf16)    # raw z transposed [d, row]
            znb = P.tile([128, NT, 128], bf16)   # normalized, transposed
            rnb = P.tile([128, NT, 128], bf16)   # rnorm broadcast to all partitions
            nrm2 = P.tile([128, NT], f32)
            nrm2g = P.tile([128, NT], f32)
            lnr = P.tile([128, NT], f32)
            rnorm = P.tile([128, NT], bf16)
            partials = P.tile([128, RT * NG], f32)
            diag = P.tile([128, RT], f32)
            pos = P.tile([128, RT], f32)
            identf = P.tile([128, 128], f32)

            nc.sync.dma_start(out=identf, in_=identf_d)

            # ---- load rows + transposed copy ----
            for t in range(NT):
                nc.sync.dma_start(out=zb[:, t, :], in_=zrv[t])
            for c in range(16):
                nc.sync.dma_start(
                    out=zt[:, 4 * c : 4 * c + 4, :],
                    in_=zt_d[:, 512 * c : 512 * (c + 1)].rearrange(
                        "d (t c) -> d t c", c=128
                    ),
                )

            # ---- row norms (DVE) + rnorm = exp(-0.5*ln(nrm2)) (ScalarE) ----
            for t in range(NT):
                sq = W.tile([128, D], bf16, tag="sq")
                nc.vector.scalar_tensor_tensor(
                    out=sq,
                    in0=zb[:, t, :],
                    scalar=1.0,
                    in1=zb[:, t, :],
                    op0=Alu.mult,
                    op1=Alu.mult,
                    accum_out=nrm2[:, t : t + 1],
                )
            nc.vector.tensor_scalar_max(out=nrm2g, in0=nrm2, scalar1=1e-16)
            nc.scalar.activation(out=lnr, in_=nrm2g, func=Act.Ln)
            nc.scalar.activation(out=rnorm, in_=lnr, func=Act.Exp, scale=-0.5)

            # ---- rnorm -> row-order in DRAM -> broadcast to 128 partitions ----
            # rnsc[t, p] = rnorm[p, t]
            nc.sync.dma_start(out=rnsc.ap().rearrange("t p -> p t"), in_=rnorm)
            rn_flat = bass.AP(
                tensor=rnsc, offset=0, ap=[[0, 128], [1, N]]
            )  # broadcast read: every partition reads the same 8192 values
            nc.sync.dma_start(out=rnb.rearrange("p t d -> p (t d)"), in_=rn_flat)

            # ---- column-normalize in transposed layout: znb = zt * rnorm[col] ----
            for c in range(16):
                nc.vector.tensor_mul(
                    znb[:, 4 * c : 4 * c + 4, :],
                    zt[:, 4 * c : 4 * c + 4, :],
                    rnb[:, 4 * c : 4 * c + 4, :],
                )

            # ---- main loop: sim row-tiles -> exp row sums (+ diag/pos taps) ----
            for rt in range(RT):
                wt = znb[:, rt, :]  # (128,128) bf16 stationary: rows rt*128..+128
                for gi in range(NG):
                    grp = G.tile([128, GW], f32, tag="grp")
                    for k in range(GW // 512):
                        c0 = gi * GW + k * 512
                        nc.tensor.matmul(
                            grp[:, k * 512 : (k + 1) * 512],
                            lhsT=wt,
                            rhs=znb[:, c0 // 128 : c0 // 128 + 4, :],
                            start=True,
                            stop=True,
                        )
                    esc = W.tile([128, GW], bf16, tag="esc")
                    nc.scalar.activation(
                        out=esc,
                        in_=grp,
                        func=Act.Exp,
                        scale=2.0,
                        accum_out=partials[:, rt * NG + gi : rt * NG + gi + 1],
                    )
                    # diagonal dot tap: col rt*128 (always group 0)
                    if gi == 0:
                        dsc = W.tile([128, 128], f32, tag="dsc")
                        nc.vector.scalar_tensor_tensor(
                            out=dsc,
                            in0=grp[:, rt * 128 : rt * 128 + 128],
                            scalar=1.0,
                            in1=identf,
                            op0=Alu.mult,
                            op1=Alu.mult,
                            accum_out=diag[:, rt : rt + 1],
                        )
                    # positive-pair dot tap: col 4096 + rt*128 (always group 2)
                    if gi == 2:
                        psc = W.tile([128, 128], f32, tag="psc")
                        nc.vector.scalar_tensor_tensor(
                            out=psc,
                            in0=grp[:, rt * 128 : rt * 128 + 128],
                            scalar=1.0,
                            in1=identf,
                            op0=Alu.mult,
                            op1=Alu.mult,
                            accum_out=pos[:, rt : rt + 1],
                        )

            # ---- epilogue: per-row loss terms ----
            rows = P.tile([128, RT], f32)
            exp2d = P.tile([128, RT], f32)
            negsum = P.tile([128, RT], f32)
            lse = P.tile([128, RT], f32)
            lossb = P.tile([128, RT], f32)
            for rt in range(RT):
                nc.vector.tensor_reduce(
                    out=rows[:, rt : rt + 1],
                    in_=partials[:, rt * NG : (rt + 1) * NG],
                    axis=mybir.AxisListType.X,
                    op=Alu.add,
                )
            nc.scalar.activation(out=exp2d, in_=diag, func=Act.Exp, scale=2.0)
            nc.vector.tensor_sub(negsum, rows, exp2d)
            nc.scalar.activation(out=lse, in_=negsum, func=Act.Ln)
            nc.vector.scalar_tensor_tensor(
                out=lossb,
                in0=pos,
                scalar=-2.0,
                in1=lse,
                op0=Alu.mult,
                op1=Alu.add,
            )
            nc.sync.dma_start(out=out_d, in_=lossb)

    nc.compile()
    return nc


def _get_nc():
    if "nc" not in _CACHE:
        _CACHE["nc"] = _build()
    return _CACHE["nc"]


def _in_maps(z_i, z_j):
    import ml_dtypes

    z = np.concatenate(
        [np.asarray(z_i, dtype=np.float32), np.asarray(z_j, dtype=np.float32)], axis=0
    )
    zb = z.astype(ml_dtypes.bfloat16)
    maps = []
    for m in range(NCORES):
        zm = np.roll(zb, -RPC * m, axis=0)
        maps.append(
            {"zr": np.ascontiguousarray(zm), "zt": np.ascontiguousarray(zm.T)}
        )
    return maps


def run(z_i: np.ndarray, z_j: np.ndarray, trace: bool = False):
    from concourse import bass_utils

    nc = _get_nc()
    res = bass_utils.run_bass_kernel_spmd(
        nc, _in_maps(z_i, z_j), core_ids=list(range(NCORES)), trace=trace
    )
    total = sum(r["out"].astype(np.float64).sum() for r in res.results)
    return np.array(total / N, dtype=np.float32), res


def kernel(z_i: np.ndarray, z_j: np.ndarray) -> np.ndarray:
    return run(z_i, z_j)[0]
